# revision 25
# baseline (speedup 1.0000x reference)
"""Trainium2 Bass kernel for nn_AdaFeatBlock (modulated deformable-conv block).

Sharding: data-parallel over batch - 8 samples -> 8 NeuronCores, all weights
replicated (host-prepacked into device-friendly layouts); each core computes
its sample end-to-end, host stacks outputs.

Per-core pipeline (one sample, x [64,128,128]):
  1. x -> bf16 "half-split" padded layout x_sb: partition h*64+c; free =
     76 stored rows (half rows -6..69) x 130 cols (-1..128), zero borders.
  2. offset/mask 3x3 conv: 9 shifted matmuls per 512-px block with a
     host-packed block-diagonal lhsT [128, 54] (row order role*18+h*9+k),
     PSUM-accumulated; each block's PSUM is DMA-scattered into math-layout
     tiles OY/OX/OM [72, 2048] (partition = chunk*18 + h*9 + k).
  3. Coordinate math on [72, 2048] tiles (all 4 pixel-chunks at once in the
     partition dim): bilinear corner weights -> W4 [72, 4qi*2048] bf16 and
     quad-table indices -> IDX [72, 2048] i16.
  4. IDX -> DRAM bounce -> idxt [128, 36*128] i16 in ap_gather stream
     layout: per (cb,k) call, partition j of each 16-partition group holds
     the indices of pixels cb*2048 + j*128 .. +127 (stream u = s*16+j).
  5. Quad gather table Q[128, 10032*4] bf16 (2x2 pixel blocks at 4 row/col
     parities, built by Act-engine strided copies); ap_gather (d=2 f32 view
     = 8B quad) fetches 2048 px * 4 corners for all 128 partitions.
  6. Per (cb,k,sub): selector matmul broadcasts W4 rows quad-minor into
     PSUM [128, 2048]; Act copies PSUM->bf16 (some subs); DVE multiplies
     with gathered quads; 4 matmuls with block-diag channel-duplicated w_dc
     accumulate over (k,qi) into po PSUM.
  7. Act adds b_dc and un-permutes stream->pixel order; DMA out.
"""

import numpy as np
import ml_dtypes

import concourse.bass as bass
import concourse.tile as tile
from concourse import mybir
from concourse.bass_utils import run_bass_kernel_spmd
from concourse import library_config
from concourse.library_overlay import lower_extended_insts
from concourse.vector_clock import ScopedClock

AF = mybir.ActivationFunctionType
ALU = mybir.AluOpType
DT = mybir.dt

B, C, H, W = 8, 64, 128, 128
O = 64
K = 3
KF = 9
NCORES = 8
HALF = H // 2
NPIX = H * W // 2              # 8192 pixels per half
ROWS_ST = 76                   # stored rows per half
PITCH = 130                    # stored cols (-1..128)
RY_N, RX_N = 38, 66
RR = RY_N * RX_N               # 2508
NBLK = 4 * RR                  # 10032
NCH = 4                        # pixel chunks per half
MC = NPIX // NCH               # 2048 px per chunk
SUB = 512
MROW = 2 * KF * NCH            # 72 math rows
MAGIC = 8388608.0              # 2^23 round-to-int magic

BF16 = ml_dtypes.bfloat16


def _install_compat():
    """This walrus build accepts at most ONE sync-wait per instruction."""
    if getattr(tile.TileContext, "_adafeat_patched", False):
        return
    _orig_lower = tile.TileContext._lower_ordered_insts

    def _split_waits(nc, ordered):
        for insts in ordered.values():
            new_insts = []
            for inst in insts:
                si = inst.sync_info
                if si is not None and si.on_wait and len(si.on_wait) > 1:
                    waits = list(si.on_wait)
                    for w in waits[:-1]:
                        nop = mybir.InstNoOp(name=f"I-{nc.next_id()}", ins=[], outs=[])
                        nop.engine = inst.engine
                        nop.sync_info = mybir.SyncInfo(on_wait=[w], on_update=[])
                        new_insts.append(nop)
                    inst.sync_info = mybir.SyncInfo(
                        on_wait=[waits[-1]], on_update=list(si.on_update)
                    )
                new_insts.append(inst)
            insts[:] = new_insts

    def _lower_split(self, ordered):
        _split_waits(self.nc, ordered)
        return _orig_lower(self, ordered)

    def _drain_split(self, tick_clock, wait_clock):
        carrier = self.nc.sync.nop(nofuse=True)
        wait_clock.add_sem_waits(
            carrier.ins, ScopedClock({None: tick_clock.global_clock})
        )
        si = carrier.ins.sync_info
        if si is not None and si.on_wait and len(si.on_wait) > 1:
            waits = list(si.on_wait)
            carrier.ins.sync_info = mybir.SyncInfo(
                on_wait=waits[:1], on_update=list(si.on_update)
            )
            for w in waits[1:]:
                extra = self.nc.sync.nop(nofuse=True)
                extra.ins.sync_info = mybir.SyncInfo(on_wait=[w], on_update=[])
        self.nc.sync.drain()
        self.nc.all_engine_barrier()
        popped = self.nc._tile_sem_poison_stack.pop()
        assert popped is self._sem_poison
        self.nc.clear_and_free_semaphores(list(self.sems.allocated().values()))
        self.nc.all_engine_barrier()

    tile.TileContext._lower_ordered_insts = _lower_split
    tile.TileContext._drain_and_barrier = _drain_split
    tile.TileContext._adafeat_patched = True


def _fap(v, extra_off, dims):
    """AP with custom free dims on an SBUF/PSUM tile view (strides in elems)."""
    return bass.AP(v.tensor, v.offset + extra_off, [v.ap[0]] + dims)


def _emit(nc, tc, ext):
    x_ext = ext["x"]
    out_ext = ext["out"]

    with tc.tile_pool(name="persist", bufs=1) as persist:
        qtab = persist.tile([128, NBLK * 4], DT.bfloat16)
        idxt = persist.tile([128, KF * NCH * 128], DT.int16)
        wdup = persist.tile([128, KF * 128], DT.bfloat16)
        lhs_om = persist.tile([128, KF * 54], DT.bfloat16)
        cst = persist.tile([MROW, 4], DT.float32)
        iot2 = persist.tile([MROW, 2 * MC], DT.bfloat16)
        bdc_t = persist.tile([128, 1], DT.float32)

        # param loads (contiguous, few big descriptors each)
        nc.sync.dma_start(out=wdup[:], in_=ext["wdup"][:])
        nc.sync.dma_start(out=lhs_om[:], in_=ext["lhs_om"][:])
        nc.sync.dma_start(out=cst[:], in_=ext["cst"][:])
        nc.sync.dma_start(out=iot2[:], in_=ext["iot2"][:])
        nc.sync.dma_start(out=bdc_t[:], in_=ext["bdc_t"][:])

        q4 = qtab[:].rearrange("p (blk q) -> p blk q", q=4)

        idx_dram = nc.dram_tensor("idx_scratch", [MROW, MC], DT.int16)
        # 64 replicas of W4 so the per-(cb,k) weight broadcast reads distinct
        # contiguous 16KB per partition (full DMA queue bandwidth)
        w4rep = nc.dram_tensor("w4rep_scratch", [64 * MROW, 4 * MC], DT.bfloat16)

        with tc.tile_pool(name="pmain", bufs=1) as pmain:
            W4 = pmain.tile([MROW, 4 * MC], DT.bfloat16)
            OY = pmain.tile([MROW, MC], DT.float32)
            OX = pmain.tile([MROW, MC], DT.float32)
            OM = pmain.tile([MROW, MC], DT.float32)
            OMs = pmain.tile([MROW, MC], DT.bfloat16)
            IDX = pmain.tile([MROW, MC], DT.int16)

            with (
                tc.tile_pool(name="px", bufs=1) as px,
                tc.tile_pool(name="convp", bufs=8, space="PSUM") as convp,
            ):
                x_sb = px.tile([128, ROWS_ST * PITCH], DT.bfloat16)
                x3 = lambda: x_sb[:].rearrange("p (r c) -> p r c", c=PITCH)

                # zero borders only: top/bottom halo rows + left/right cols
                nc.vector.memset(x3()[0:64, 0:6, :], 0.0)
                nc.vector.memset(x3()[64:128, 70:76, :], 0.0)
                nc.vector.memset(x3()[:, :, 0:1], 0.0)
                nc.vector.memset(x3()[:, :, 129:130], 0.0)
                # qtab memset on DVE (keeps the gpsimd queue free for x DMAs)
                nc.vector.memset(qtab[:], 0.0)

                xv = x_ext[:]
                for h in range(2):
                    r0 = max(0, h * HALF - 6)
                    r1 = min(H - 1, h * HALF + 69)
                    nrow = r1 - r0 + 1
                    rloc = r0 - (h * HALF - 6)
                    dst = x3()[h * 64 : h * 64 + 64, rloc : rloc + nrow, 1 : 1 + W]
                    nc.gpsimd.dma_start(out=dst, in_=xv[:, r0 : r1 + 1, :])

                _lib = nc.gpsimd.load_library(library_config.ap_gather)

                # ---- offset/mask conv: 2 passes x 8 blocks, tap-outer ----
                for grp in range(2):
                    pts = [
                        convp.tile([54, SUB], DT.float32, tag="cpt", name=f"cpt{b}")
                        for b in range(8)
                    ]
                    for i in range(KF):
                        dy, dx = i // 3, i % 3
                        for bi in range(8):
                            blk = grp * 8 + bi
                            r0 = blk * 4
                            rhs = x3()[:, 6 + r0 + dy - 1 : 6 + r0 + dy + 3,
                                       dx : dx + W]
                            nc.tensor.matmul(
                                out=pts[bi][:],
                                lhsT=lhs_om[:, i * 54 : (i + 1) * 54],
                                rhs=rhs,
                                start=(i == 0), stop=(i == KF - 1),
                            )
                    for bi in range(8):
                        blk = grp * 8 + bi
                        cb2, po_ = blk // 4, (blk % 4) * SUB
                        ob = px.tile([54, SUB], DT.float32, tag="ob", name="ob",
                                     bufs=4)
                        nc.scalar.activation(out=ob[:], in_=pts[bi][:], func=AF.Copy)
                        for role, dstt in ((0, OY), (1, OX), (2, OM)):
                            nc.sync.dma_start(
                                out=dstt[cb2 * 18 : cb2 * 18 + 18, po_ : po_ + SUB],
                                in_=ob[role * 18 : role * 18 + 18, :],
                            )

                # ---- quad gather table from x_sb (Act engine copies) ----
                for a in range(2):
                    for b in range(2):
                        blk0 = (a * 2 + b) * RR
                        for qy in range(2):
                            for qx in range(2):
                                ry_cnt = min((75 - a - qy) // 2 + 1, RY_N)
                                rx0 = 1 if (b + qx) == 0 else 0
                                rx1 = min(RX_N - 1, (130 - b - qx) // 2)
                                rx_cnt = rx1 - rx0 + 1
                                c0 = 2 * rx0 + b + qx - 1
                                src = x3()[:, a + qy : a + qy + 2 * (ry_cnt - 1) + 1 : 2,
                                           c0 : c0 + 2 * (rx_cnt - 1) + 1 : 2]
                                dst3 = q4[:, blk0 + rx0 : blk0 + rx0
                                          + (ry_cnt - 1) * RX_N + rx_cnt,
                                          qy * 2 + qx : qy * 2 + qx + 1]
                                dst = bass.AP(
                                    dst3.tensor, dst3.offset,
                                    [dst3.ap[0], [RX_N * 4, ry_cnt], [4, rx_cnt]],
                                )
                                if qy == 0:
                                    nc.scalar.activation(out=dst, in_=src,
                                                         func=AF.Copy)
                                else:
                                    nc.vector.tensor_copy(out=dst, in_=src)

            # ---- coordinate math on [72, 2048] ----
            with tc.tile_pool(name="ptmp", bufs=1) as ptmp:
                TA = ptmp.tile([MROW, MC], DT.float32)
                TB = ptmp.tile([MROW, MC], DT.float32)
                TC_ = ptmp.tile([MROW, MC], DT.float32)
                TD = ptmp.tile([MROW, MC], DT.float32)

                ts = nc.vector.tensor_scalar
                tt = nc.vector.tensor_tensor
                stt = nc.vector.scalar_tensor_tensor

                # mask = sigmoid(om_m + b_om_m) on Act (x2 folded into wdup)
                nc.scalar.activation(out=OMs[:], in_=OM[:], func=AF.Sigmoid,
                                     bias=cst[:, 2:3], scale=1.0)

                # y-pass: P = OY + cst_y + iota_row
                stt(out=TA[:], in0=OY[:], scalar=cst[:, 0:1], in1=iot2[:, 0:MC],
                    op0=ALU.add, op1=ALU.add)
                ts(out=TB[:], in0=TA[:], scalar1=MAGIC, scalar2=-MAGIC,
                   op0=ALU.add, op1=ALU.add)
                tt(out=TC_[:], in0=TB[:], in1=TA[:], op=ALU.is_gt)
                tt(out=OY[:], in0=TB[:], in1=TC_[:], op=ALU.subtract)   # y0_local
                tt(out=TB[:], in0=TA[:], in1=OY[:], op=ALU.subtract)    # fy
                ts(out=OY[:], in0=OY[:], scalar1=0.0, scalar2=75.0,
                   op0=ALU.max, op1=ALU.min)
                nc.vector.tensor_scalar_mul(out=TA[:], in0=OY[:], scalar1=0.5)
                ts(out=TC_[:], in0=TA[:], scalar1=MAGIC, scalar2=-MAGIC,
                   op0=ALU.add, op1=ALU.add)
                tt(out=OY[:], in0=TC_[:], in1=TA[:], op=ALU.is_gt)
                tt(out=TC_[:], in0=TC_[:], in1=OY[:], op=ALU.subtract)  # ry
                tt(out=TA[:], in0=TA[:], in1=TC_[:], op=ALU.subtract)   # pa_y/2

                # x-pass: P = OX + cst_x + iota_col  (value = x0_stored+1 dance)
                stt(out=TD[:], in0=OX[:], scalar=cst[:, 1:2], in1=iot2[:, MC : 2 * MC],
                    op0=ALU.add, op1=ALU.add)
                ts(out=OX[:], in0=TD[:], scalar1=MAGIC, scalar2=-MAGIC,
                   op0=ALU.add, op1=ALU.add)
                tt(out=OM[:], in0=OX[:], in1=TD[:], op=ALU.is_gt)
                tt(out=OX[:], in0=OX[:], in1=OM[:], op=ALU.subtract)    # x0_stored+1
                tt(out=TD[:], in0=TD[:], in1=OX[:], op=ALU.subtract)    # fx
                ts(out=OX[:], in0=OX[:], scalar1=0.0, scalar2=130.0,
                   op0=ALU.max, op1=ALU.min)
                nc.vector.tensor_scalar_mul(out=OM[:], in0=OX[:], scalar1=0.5)
                ts(out=OX[:], in0=OM[:], scalar1=MAGIC, scalar2=-MAGIC,
                   op0=ALU.add, op1=ALU.add)
                tt(out=OY[:], in0=OX[:], in1=OM[:], op=ALU.is_gt)
                tt(out=OX[:], in0=OX[:], in1=OY[:], op=ALU.subtract)    # rx
                tt(out=OM[:], in0=OM[:], in1=OX[:], op=ALU.subtract)    # pa_x/2

                # idx = pa_y*4RR + pa_x*2RR + ry*RX_N + rx  (pa_* are half-parities)
                stt(out=OY[:], in0=TC_[:], scalar=float(RX_N), in1=OX[:],
                    op0=ALU.mult, op1=ALU.add)
                stt(out=TC_[:], in0=TA[:], scalar=float(4 * RR), in1=OY[:],
                    op0=ALU.mult, op1=ALU.add)
                stt(out=OY[:], in0=OM[:], scalar=float(2 * RR), in1=TC_[:],
                    op0=ALU.mult, op1=ALU.add)
                nc.vector.tensor_copy(out=IDX[:], in_=OY[:])

                # idx bounce: SBUF -> DRAM -> stream-layout idxt
                nc.sync.dma_start(out=idx_dram[:], in_=IDX[:])
                dv = idx_dram[:]
                for h in range(2):
                    for g in range(4):
                        p0 = h * 64 + g * 16
                        for cb in range(NCH):
                            src = bass.AP(
                                dv.tensor, dv.offset + (cb * 18 + h * KF) * MC,
                                [[128, 16], [MC, KF], [1, 128]],
                            )
                            dst = idxt[p0 : p0 + 16,
                                       cb * KF * 128 : (cb + 1) * KF * 128
                                       ].rearrange("p (k s) -> p k s", k=KF)
                            nc.sync.dma_start(out=dst, in_=src)

                # corner weights -> W4 (quad-minor per qi block)
                nc.scalar.activation(out=TC_[:], in_=TB[:], func=AF.Copy,
                                     bias=1.0, scale=-1.0)               # 1-fy
                nc.scalar.activation(out=OX[:], in_=TD[:], func=AF.Copy,
                                     bias=1.0, scale=-1.0)               # 1-fx
                # W4 in stream-quad-minor order: col = (s*16+j)*4 + qi for
                # pixel q = j*128+s  (strided DVE write, everything downstream
                # then reads contiguously)
                w4w = W4[:]
                for qy in range(2):
                    for qx in range(2):
                        qi = qy * 2 + qx
                        ya = TB if qy else TC_
                        xa = TD if qx else OX
                        tt(out=OY[:], in0=ya[:], in1=xa[:], op=ALU.mult)
                        tt(out=bass.AP(w4w.tensor, w4w.offset + qi,
                                       [w4w.ap[0], [4, 16], [64, 128]]),
                           in0=OY[:], in1=OMs[:], op=ALU.mult)
                # replicate x64 into DRAM, grouped by chunk so phase 4 can
                # start as soon as its chunk's replicas land
                for cb in range(NCH):
                    for r in range(64):
                        eng = (nc.sync, nc.gpsimd)[r % 2]
                        eng.dma_start(
                            out=w4rep[r * MROW + cb * 18 : r * MROW + cb * 18 + 18, :],
                            in_=W4[cb * 18 : cb * 18 + 18, :],
                        )

        # ---- phase 4: gather + DMA-broadcast weights + modulate + matmul ----
        qtab_f32 = qtab[:].bitcast(DT.float32)
        outv = out_ext[:].rearrange("o h w -> o (h w)")
        w4v = w4rep[:]

        with (
            tc.tile_pool(name="g", bufs=2) as gpool,
            tc.tile_pool(name="mqs", bufs=2) as mqsp,
            tc.tile_pool(name="ht", bufs=2) as htp,
            tc.tile_pool(name="ot", bufs=2) as otp,
            tc.tile_pool(name="op", bufs=2, space="PSUM") as opsum,
        ):
            for cb in range(NCH):
                po = opsum.tile([128, MC], DT.float32, tag="po")
                for k in range(KF):
                    # weight broadcast: partition h*64+r reads row
                    # (r*72 + cb*18 + h*9 + k) of w4rep - one call, 128
                    # distinct contiguous 16KB descriptors
                    mqs = mqsp.tile([128, 4 * MC], DT.bfloat16, tag="mqs")
                    for i in range(16):
                        h, r0 = i // 8, (i % 8) * 8
                        src = bass.AP(
                            w4v.tensor,
                            w4v.offset
                            + (r0 * MROW + cb * 18 + h * KF + k) * 4 * MC,
                            [[MROW * 4 * MC, 8], [1, 4 * MC]],
                        )
                        eng = nc.sync if i % 2 else nc.gpsimd
                        eng.dma_start(
                            out=mqs[h * 64 + r0 : h * 64 + r0 + 8, :], in_=src)
                    g = gpool.tile([128, MC * 2], DT.float32, tag="g")
                    idx_sl = idxt[:, (cb * KF + k) * 128 : (cb * KF + k + 1) * 128]
                    _ga = nc.gpsimd.ap_gather(
                        g[:], qtab_f32, idx_sl,
                        channels=128, num_elems=NBLK, d=2, num_idxs=MC,
                    )
                    tile.add_dep_helper(_ga.ins, _lib.ins, reason="gather after lib")
                    gb = g[:].bitcast(DT.bfloat16)

                    for sub in range(4):
                        # all-contiguous modulate: stream-quad-minor everywhere
                        ht = htp.tile([128, 2048], DT.bfloat16, tag="ht")
                        nc.vector.tensor_tensor(
                            out=ht[:],
                            in0=mqs[:, sub * 2048 : (sub + 1) * 2048],
                            in1=gb[:, sub * 2048 : (sub + 1) * 2048],
                            op=ALU.mult)
                        htv = ht[:]
                        for qi in range(4):
                            nc.tensor.matmul(
                                out=po[:, sub * SUB : (sub + 1) * SUB],
                                lhsT=wdup[:, k * 128 : (k + 1) * 128],
                                rhs=bass.AP(htv.tensor, htv.offset + qi,
                                            [htv.ap[0], [4, 512]]),
                                start=(k == 0 and qi == 0),
                                stop=(k == KF - 1 and qi == 3),
                            )

                # bias add + stream->pixel unpermute, then store
                ot2 = otp.tile([128, MC], DT.float32, tag="ot")
                pov = po[:]
                nc.scalar.activation(
                    out=ot2[:],
                    in_=bass.AP(pov.tensor, pov.offset,
                                [pov.ap[0], [1, 16], [16, 128]]),
                    func=AF.Identity, bias=bdc_t[:, 0:1], scale=1.0)
                for h in range(2):
                    nc.sync.dma_start(
                        out=outv[:, h * NPIX + cb * MC : h * NPIX + (cb + 1) * MC],
                        in_=ot2[h * 64 : h * 64 + 64, :],
                    )


def _pack_params(w_om, b_om, w_dc, b_dc):
    # conv lhsT tiles [128, 9*54]: row h*64+c, col dd*54 + (role*18 + h*9 + k)
    lhs = np.zeros((128, KF * 54), np.float32)
    for dd in range(KF):
        dy, dx = dd // 3, dd % 3
        for h in range(2):
            for kk in range(KF):
                lhs[h * 64 : h * 64 + 64, dd * 54 + 0 * 18 + h * 9 + kk] = \
                    w_om[2 * kk, :, dy, dx]
                lhs[h * 64 : h * 64 + 64, dd * 54 + 1 * 18 + h * 9 + kk] = \
                    w_om[2 * kk + 1, :, dy, dx]
                lhs[h * 64 : h * 64 + 64, dd * 54 + 2 * 18 + h * 9 + kk] = \
                    w_om[18 + kk, :, dy, dx]

    # wdup [128, 9*128] with the mask's 2.0 folded in
    wd = np.zeros((128, KF * 128), np.float32)
    for k in range(KF):
        kh, kw = k // 3, k % 3
        for h in range(2):
            wd[h * 64 : h * 64 + 64, k * 128 + h * 64 : k * 128 + h * 64 + 64] = \
                2.0 * w_dc[:, :, kh, kw].T

    # per-row constants [72, 4]: cst_y, cst_x, mask bias
    cstv = np.zeros((MROW, 4), np.float32)
    for p in range(MROW):
        cb, hk = divmod(p, 18)
        h, kk = divmod(hk, 9)
        kh, kw = kk // 3, kk % 3
        cstv[p, 0] = b_om[2 * kk] + cb * 16 + kh + 5
        cstv[p, 1] = b_om[2 * kk + 1] + kw + 1
        cstv[p, 2] = b_om[18 + kk]

    # iotas [72, 2*2048]: row (u//128) then col (u%128); exact in bf16
    u = np.arange(MC)
    iot = np.concatenate([u // 128, u % 128]).astype(np.float32)
    iot2 = np.broadcast_to(iot, (MROW, 2 * MC)).copy()

    bdc = np.zeros((128, 1), np.float32)
    bdc[:64, 0] = b_dc
    bdc[64:, 0] = b_dc

    return {
        "lhs_om": lhs.astype(BF16),
        "wdup": wd.astype(BF16),
        "cst": cstv,
        "iot2": iot2.astype(BF16),
        "bdc_t": bdc,
    }


def _build_nc():
    _install_compat()
    nc = bass.Bass()
    ext = {}
    ext["x"] = nc.declare_dram_parameter("x", [C, H, W], DT.float32, isOutput=False)
    ext["lhs_om"] = nc.declare_dram_parameter("lhs_om", [128, KF * 54], DT.bfloat16, isOutput=False)
    ext["wdup"] = nc.declare_dram_parameter("wdup", [128, KF * 128], DT.bfloat16, isOutput=False)
    ext["cst"] = nc.declare_dram_parameter("cst", [MROW, 4], DT.float32, isOutput=False)
    ext["iot2"] = nc.declare_dram_parameter("iot2", [MROW, 2 * MC], DT.bfloat16, isOutput=False)
    ext["bdc_t"] = nc.declare_dram_parameter("bdc_t", [128, 1], DT.float32, isOutput=False)
    ext["out"] = nc.declare_dram_parameter("out", [O, H, W], DT.float32, isOutput=True)
    with tile.TileContext(nc) as tc:
        _emit(nc, tc, ext)
    lower_extended_insts(nc)
    return nc


_NC_CACHE = None


def kernel(**inputs):
    global _NC_CACHE
    x = np.ascontiguousarray(inputs["x"], dtype=np.float32)
    w_om = np.ascontiguousarray(inputs["w_om"], dtype=np.float32)
    b_om = np.ascontiguousarray(inputs["b_om"], dtype=np.float32)
    w_dc = np.ascontiguousarray(inputs["w_dc"], dtype=np.float32)
    b_dc = np.ascontiguousarray(inputs["b_dc"], dtype=np.float32)

    if _NC_CACHE is None:
        _NC_CACHE = _build_nc()
    nc = _NC_CACHE

    packed = _pack_params(w_om, b_om, w_dc, b_dc)
    in_maps = [{"x": x[i], **packed} for i in range(NCORES)]
    res = run_bass_kernel_spmd(nc, in_maps, core_ids=list(range(NCORES)))
    return np.stack(
        [np.asarray(res.results[i]["out"]) for i in range(NCORES)]
    ).astype(np.float32)


# revision 29
# speedup vs baseline: 1.2932x; 1.2932x over previous
"""Trainium2 Bass kernel for nn_AdaFeatBlock (modulated deformable-conv block).

Sharding: data-parallel over batch - 8 samples -> 8 NeuronCores, all weights
replicated (host-prepacked into device-friendly layouts); each core computes
its sample end-to-end, host stacks outputs.

Per-core pipeline (one sample, x [64,128,128]):
  1. x -> bf16 "half-split" padded layout x_sb: partition h*64+c; free =
     76 stored rows (half rows -6..69) x 130 cols (-1..128), zero borders.
  2. offset/mask 3x3 conv: 9 shifted matmuls per 512-px block with a
     host-packed block-diagonal lhsT [128, 54] (row order role*18+h*9+k),
     PSUM-accumulated; each block's PSUM is DMA-scattered into math-layout
     tiles OY/OX/OM [72, 2048] (partition = chunk*18 + h*9 + k).
  3. Coordinate math on [72, 2048] tiles (all 4 pixel-chunks at once in the
     partition dim): bilinear corner weights -> W4 [72, 4qi*2048] bf16 and
     quad-table indices -> IDX [72, 2048] i16.
  4. IDX -> DRAM bounce -> idxt [128, 36*128] i16 in ap_gather stream
     layout: per (cb,k) call, partition j of each 16-partition group holds
     the indices of pixels cb*2048 + j*128 .. +127 (stream u = s*16+j).
  5. Quad gather table Q[128, 10032*4] bf16 (2x2 pixel blocks at 4 row/col
     parities, built by Act-engine strided copies); ap_gather (d=2 f32 view
     = 8B quad) fetches 2048 px * 4 corners for all 128 partitions.
  6. Per (cb,k,sub): selector matmul broadcasts W4 rows quad-minor into
     PSUM [128, 2048]; Act copies PSUM->bf16 (some subs); DVE multiplies
     with gathered quads; 4 matmuls with block-diag channel-duplicated w_dc
     accumulate over (k,qi) into po PSUM.
  7. Act adds b_dc and un-permutes stream->pixel order; DMA out.
"""

import numpy as np
import ml_dtypes

import concourse.bass as bass
import concourse.tile as tile
from concourse import mybir
from concourse.bass_utils import run_bass_kernel_spmd
from concourse import library_config
from concourse.library_overlay import lower_extended_insts
from concourse.vector_clock import ScopedClock

AF = mybir.ActivationFunctionType
ALU = mybir.AluOpType
DT = mybir.dt

B, C, H, W = 8, 64, 128, 128
O = 64
K = 3
KF = 9
NCORES = 8
HALF = H // 2
NPIX = H * W // 2              # 8192 pixels per half
ROWS_ST = 76                   # stored rows per half
PITCH = 130                    # stored cols (-1..128)
RY_N, RX_N = 38, 66
RR = RY_N * RX_N               # 2508
NBLK = 4 * RR                  # 10032
NCH = 4                        # pixel chunks per half
MC = NPIX // NCH               # 2048 px per chunk
SUB = 512
MROW = 2 * KF * NCH            # 72 math rows
MAGIC = 8388608.0              # 2^23 round-to-int magic

BF16 = ml_dtypes.bfloat16


def _install_compat():
    """This walrus build accepts at most ONE sync-wait per instruction."""
    if getattr(tile.TileContext, "_adafeat_patched", False):
        return
    _orig_lower = tile.TileContext._lower_ordered_insts

    def _split_waits(nc, ordered):
        for insts in ordered.values():
            new_insts = []
            for inst in insts:
                si = inst.sync_info
                if si is not None and si.on_wait and len(si.on_wait) > 1:
                    waits = list(si.on_wait)
                    for w in waits[:-1]:
                        nop = mybir.InstNoOp(name=f"I-{nc.next_id()}", ins=[], outs=[])
                        nop.engine = inst.engine
                        nop.sync_info = mybir.SyncInfo(on_wait=[w], on_update=[])
                        new_insts.append(nop)
                    inst.sync_info = mybir.SyncInfo(
                        on_wait=[waits[-1]], on_update=list(si.on_update)
                    )
                new_insts.append(inst)
            insts[:] = new_insts

    def _lower_split(self, ordered):
        _split_waits(self.nc, ordered)
        return _orig_lower(self, ordered)

    def _drain_split(self, tick_clock, wait_clock):
        carrier = self.nc.sync.nop(nofuse=True)
        wait_clock.add_sem_waits(
            carrier.ins, ScopedClock({None: tick_clock.global_clock})
        )
        si = carrier.ins.sync_info
        if si is not None and si.on_wait and len(si.on_wait) > 1:
            waits = list(si.on_wait)
            carrier.ins.sync_info = mybir.SyncInfo(
                on_wait=waits[:1], on_update=list(si.on_update)
            )
            for w in waits[1:]:
                extra = self.nc.sync.nop(nofuse=True)
                extra.ins.sync_info = mybir.SyncInfo(on_wait=[w], on_update=[])
        self.nc.sync.drain()
        self.nc.all_engine_barrier()
        popped = self.nc._tile_sem_poison_stack.pop()
        assert popped is self._sem_poison
        self.nc.clear_and_free_semaphores(list(self.sems.allocated().values()))
        self.nc.all_engine_barrier()

    tile.TileContext._lower_ordered_insts = _lower_split
    tile.TileContext._drain_and_barrier = _drain_split
    tile.TileContext._adafeat_patched = True


def _fap(v, extra_off, dims):
    """AP with custom free dims on an SBUF/PSUM tile view (strides in elems)."""
    return bass.AP(v.tensor, v.offset + extra_off, [v.ap[0]] + dims)


def _emit(nc, tc, ext):
    x_ext = ext["x"]
    out_ext = ext["out"]

    with tc.tile_pool(name="persist", bufs=1) as persist:
        qtab = persist.tile([128, NBLK * 4], DT.bfloat16)
        idxt = persist.tile([128, KF * NCH * 128], DT.int16)
        wdup = persist.tile([128, KF * 128], DT.bfloat16)
        sel36 = persist.tile([MROW, KF * NCH * 128], DT.bfloat16)
        lhs_om = persist.tile([128, KF * 54], DT.bfloat16)
        cst = persist.tile([MROW, 4], DT.float32)
        iot2 = persist.tile([MROW, 2 * MC], DT.bfloat16)
        bdc_t = persist.tile([128, 1], DT.float32)
        W4 = persist.tile([MROW, 4 * MC], DT.bfloat16)

        # param loads (contiguous, few big descriptors each)
        nc.sync.dma_start(out=wdup[:], in_=ext["wdup"][:])
        nc.sync.dma_start(out=sel36[:], in_=ext["sel36"][:])
        nc.sync.dma_start(out=lhs_om[:], in_=ext["lhs_om"][:])
        nc.sync.dma_start(out=cst[:], in_=ext["cst"][:])
        nc.sync.dma_start(out=iot2[:], in_=ext["iot2"][:])
        nc.sync.dma_start(out=bdc_t[:], in_=ext["bdc_t"][:])

        q4 = qtab[:].rearrange("p (blk q) -> p blk q", q=4)

        idx_dram = nc.dram_tensor("idx_scratch", [MROW, MC], DT.int16)

        with tc.tile_pool(name="pmain", bufs=1) as pmain:
            OY = pmain.tile([MROW, MC], DT.float32)
            OX = pmain.tile([MROW, MC], DT.float32)
            OM = pmain.tile([MROW, MC], DT.float32)
            OMs = pmain.tile([MROW, MC], DT.bfloat16)
            IDX = pmain.tile([MROW, MC], DT.int16)

            with (
                tc.tile_pool(name="px", bufs=1) as px,
                tc.tile_pool(name="convp", bufs=8, space="PSUM") as convp,
            ):
                x_sb = px.tile([128, ROWS_ST * PITCH], DT.bfloat16)
                x3 = lambda: x_sb[:].rearrange("p (r c) -> p r c", c=PITCH)

                # zero borders only: top/bottom halo rows + left/right cols
                nc.vector.memset(x3()[0:64, 0:6, :], 0.0)
                nc.vector.memset(x3()[64:128, 70:76, :], 0.0)
                nc.vector.memset(x3()[:, :, 0:1], 0.0)
                nc.vector.memset(x3()[:, :, 129:130], 0.0)
                # qtab memset on DVE (keeps the gpsimd queue free for x DMAs)
                nc.vector.memset(qtab[:], 0.0)

                xv = x_ext[:]
                for h in range(2):
                    r0 = max(0, h * HALF - 6)
                    r1 = min(H - 1, h * HALF + 69)
                    nrow = r1 - r0 + 1
                    rloc = r0 - (h * HALF - 6)
                    dst = x3()[h * 64 : h * 64 + 64, rloc : rloc + nrow, 1 : 1 + W]
                    nc.gpsimd.dma_start(out=dst, in_=xv[:, r0 : r1 + 1, :])

                _lib = nc.gpsimd.load_library(library_config.ap_gather)

                # ---- offset/mask conv: 2 passes x 8 blocks, tap-outer ----
                for grp in range(2):
                    pts = [
                        convp.tile([54, SUB], DT.float32, tag="cpt", name=f"cpt{b}")
                        for b in range(8)
                    ]
                    for i in range(KF):
                        dy, dx = i // 3, i % 3
                        for bi in range(8):
                            blk = grp * 8 + bi
                            r0 = blk * 4
                            rhs = x3()[:, 6 + r0 + dy - 1 : 6 + r0 + dy + 3,
                                       dx : dx + W]
                            nc.tensor.matmul(
                                out=pts[bi][:],
                                lhsT=lhs_om[:, i * 54 : (i + 1) * 54],
                                rhs=rhs,
                                start=(i == 0), stop=(i == KF - 1),
                            )
                    for bi in range(8):
                        blk = grp * 8 + bi
                        cb2, po_ = blk // 4, (blk % 4) * SUB
                        ob = px.tile([54, SUB], DT.float32, tag="ob", name="ob",
                                     bufs=4)
                        nc.scalar.activation(out=ob[:], in_=pts[bi][:], func=AF.Copy)
                        for role, dstt in ((0, OY), (1, OX), (2, OM)):
                            nc.sync.dma_start(
                                out=dstt[cb2 * 18 : cb2 * 18 + 18, po_ : po_ + SUB],
                                in_=ob[role * 18 : role * 18 + 18, :],
                            )

                # ---- quad gather table from x_sb (Act engine copies) ----
                for a in range(2):
                    for b in range(2):
                        blk0 = (a * 2 + b) * RR
                        for qy in range(2):
                            for qx in range(2):
                                ry_cnt = min((75 - a - qy) // 2 + 1, RY_N)
                                rx0 = 1 if (b + qx) == 0 else 0
                                rx1 = min(RX_N - 1, (130 - b - qx) // 2)
                                rx_cnt = rx1 - rx0 + 1
                                c0 = 2 * rx0 + b + qx - 1
                                src = x3()[:, a + qy : a + qy + 2 * (ry_cnt - 1) + 1 : 2,
                                           c0 : c0 + 2 * (rx_cnt - 1) + 1 : 2]
                                dst3 = q4[:, blk0 + rx0 : blk0 + rx0
                                          + (ry_cnt - 1) * RX_N + rx_cnt,
                                          qy * 2 + qx : qy * 2 + qx + 1]
                                dst = bass.AP(
                                    dst3.tensor, dst3.offset,
                                    [dst3.ap[0], [RX_N * 4, ry_cnt], [4, rx_cnt]],
                                )
                                if qy == 0:
                                    nc.scalar.activation(out=dst, in_=src,
                                                         func=AF.Copy)
                                else:
                                    nc.vector.tensor_copy(out=dst, in_=src)

            # ---- coordinate math on [72, 2048] ----
            with tc.tile_pool(name="ptmp", bufs=1) as ptmp:
                TA = ptmp.tile([MROW, MC], DT.float32)
                TB = ptmp.tile([MROW, MC], DT.float32)
                TC_ = ptmp.tile([MROW, MC], DT.float32)
                TD = ptmp.tile([MROW, MC], DT.float32)

                ts = nc.vector.tensor_scalar
                tt = nc.vector.tensor_tensor
                stt = nc.vector.scalar_tensor_tensor

                # mask = sigmoid(om_m + b_om_m) on Act (x2 folded into wdup)
                nc.scalar.activation(out=OMs[:], in_=OM[:], func=AF.Sigmoid,
                                     bias=cst[:, 2:3], scale=1.0)

                # y-pass: P = OY + cst_y + iota_row
                stt(out=TA[:], in0=OY[:], scalar=cst[:, 0:1], in1=iot2[:, 0:MC],
                    op0=ALU.add, op1=ALU.add)
                ts(out=TB[:], in0=TA[:], scalar1=MAGIC, scalar2=-MAGIC,
                   op0=ALU.add, op1=ALU.add)
                tt(out=TC_[:], in0=TB[:], in1=TA[:], op=ALU.is_gt)
                tt(out=OY[:], in0=TB[:], in1=TC_[:], op=ALU.subtract)   # y0_local
                tt(out=TB[:], in0=TA[:], in1=OY[:], op=ALU.subtract)    # fy
                ts(out=OY[:], in0=OY[:], scalar1=0.0, scalar2=75.0,
                   op0=ALU.max, op1=ALU.min)
                nc.vector.tensor_scalar_mul(out=TA[:], in0=OY[:], scalar1=0.5)
                ts(out=TC_[:], in0=TA[:], scalar1=MAGIC, scalar2=-MAGIC,
                   op0=ALU.add, op1=ALU.add)
                tt(out=OY[:], in0=TC_[:], in1=TA[:], op=ALU.is_gt)
                tt(out=TC_[:], in0=TC_[:], in1=OY[:], op=ALU.subtract)  # ry
                tt(out=TA[:], in0=TA[:], in1=TC_[:], op=ALU.subtract)   # pa_y/2

                # x-pass: P = OX + cst_x + iota_col  (value = x0_stored+1 dance)
                stt(out=TD[:], in0=OX[:], scalar=cst[:, 1:2], in1=iot2[:, MC : 2 * MC],
                    op0=ALU.add, op1=ALU.add)
                ts(out=OX[:], in0=TD[:], scalar1=MAGIC, scalar2=-MAGIC,
                   op0=ALU.add, op1=ALU.add)
                tt(out=OM[:], in0=OX[:], in1=TD[:], op=ALU.is_gt)
                tt(out=OX[:], in0=OX[:], in1=OM[:], op=ALU.subtract)    # x0_stored+1
                tt(out=TD[:], in0=TD[:], in1=OX[:], op=ALU.subtract)    # fx
                ts(out=OX[:], in0=OX[:], scalar1=0.0, scalar2=130.0,
                   op0=ALU.max, op1=ALU.min)
                nc.vector.tensor_scalar_mul(out=OM[:], in0=OX[:], scalar1=0.5)
                ts(out=OX[:], in0=OM[:], scalar1=MAGIC, scalar2=-MAGIC,
                   op0=ALU.add, op1=ALU.add)
                tt(out=OY[:], in0=OX[:], in1=OM[:], op=ALU.is_gt)
                tt(out=OX[:], in0=OX[:], in1=OY[:], op=ALU.subtract)    # rx
                tt(out=OM[:], in0=OM[:], in1=OX[:], op=ALU.subtract)    # pa_x/2

                # idx = pa_y*4RR + pa_x*2RR + ry*RX_N + rx  (pa_* are half-parities)
                stt(out=OY[:], in0=TC_[:], scalar=float(RX_N), in1=OX[:],
                    op0=ALU.mult, op1=ALU.add)
                stt(out=TC_[:], in0=TA[:], scalar=float(4 * RR), in1=OY[:],
                    op0=ALU.mult, op1=ALU.add)
                stt(out=OY[:], in0=OM[:], scalar=float(2 * RR), in1=TC_[:],
                    op0=ALU.mult, op1=ALU.add)
                nc.vector.tensor_copy(out=IDX[:], in_=OY[:])

                # idx bounce: SBUF -> DRAM -> stream-layout idxt
                nc.sync.dma_start(out=idx_dram[:], in_=IDX[:])
                dv = idx_dram[:]
                for h in range(2):
                    for g in range(4):
                        p0 = h * 64 + g * 16
                        for cb in range(NCH):
                            src = bass.AP(
                                dv.tensor, dv.offset + (cb * 18 + h * KF) * MC,
                                [[128, 16], [MC, KF], [1, 128]],
                            )
                            dst = idxt[p0 : p0 + 16,
                                       cb * KF * 128 : (cb + 1) * KF * 128
                                       ].rearrange("p (k s) -> p k s", k=KF)
                            nc.sync.dma_start(out=dst, in_=src)

                # corner weights -> W4 (quad-minor per qi block)
                nc.scalar.activation(out=TC_[:], in_=TB[:], func=AF.Copy,
                                     bias=1.0, scale=-1.0)               # 1-fy
                nc.scalar.activation(out=OX[:], in_=TD[:], func=AF.Copy,
                                     bias=1.0, scale=-1.0)               # 1-fx
                # W4 in stream-quad-minor order: col = (s*16+j)*4 + qi for
                # pixel q = j*128+s  (strided DVE write, everything downstream
                # then reads contiguously)
                w4w = W4[:]
                for qy in range(2):
                    for qx in range(2):
                        qi = qy * 2 + qx
                        ya = TB if qy else TC_
                        xa = TD if qx else OX
                        tt(out=OY[:], in0=ya[:], in1=xa[:], op=ALU.mult)
                        tt(out=bass.AP(w4w.tensor, w4w.offset + qi,
                                       [w4w.ap[0], [4, 16], [64, 128]]),
                           in0=OY[:], in1=OMs[:], op=ALU.mult)

        # ---- phase 4: gather + PE-broadcast weights + modulate + matmul ----
        qtab_f32 = qtab[:].bitcast(DT.float32)
        outv = out_ext[:].rearrange("o h w -> o (h w)")

        with (
            tc.tile_pool(name="g", bufs=2) as gpool,
            tc.tile_pool(name="mqs", bufs=2) as mqsp,
            tc.tile_pool(name="ht", bufs=2) as htp,
            tc.tile_pool(name="vt", bufs=2) as vtp,
            tc.tile_pool(name="ot", bufs=2) as otp,
            tc.tile_pool(name="mp", bufs=1, space="PSUM") as mpsum,
            tc.tile_pool(name="op", bufs=1, space="PSUM") as opsum,
        ):
            def emit_bcast(cb, k, sub):
                # mq[128, 2048] = W4 rows (cb,*,k) broadcast, stream-quad-minor
                mq = mpsum.tile([128, 2048], DT.float32, tag="mq", name="mq")
                sel_sl = sel36[:, (cb * KF + k) * 128 : (cb * KF + k + 1) * 128]
                for mm in range(4):
                    nc.tensor.matmul(
                        out=mq[:, mm * 512 : (mm + 1) * 512],
                        lhsT=sel_sl,
                        rhs=W4[:, sub * 2048 + mm * 512 : sub * 2048 + (mm + 1) * 512],
                        start=True, stop=True,
                    )
                return mq

            for cb in range(NCH):
                po = opsum.tile([128, MC], DT.float32, tag="po")
                mq_next = None
                for k in range(KF):
                    g = gpool.tile([128, MC * 2], DT.float32, tag="g")
                    idx_sl = idxt[:, (cb * KF + k) * 128 : (cb * KF + k + 1) * 128]
                    _ga = nc.gpsimd.ap_gather(
                        g[:], qtab_f32, idx_sl,
                        channels=128, num_elems=NBLK, d=2, num_idxs=MC,
                    )
                    tile.add_dep_helper(_ga.ins, _lib.ins, reason="gather after lib")
                    gb = g[:].bitcast(DT.bfloat16)

                    for sub in range(4):
                        mq = mq_next if mq_next is not None else emit_bcast(cb, k, sub)
                        mq_next = None
                        # Act evicts PSUM to bf16 so the mult runs 2x
                        mqs = mqsp.tile([128, 2048], DT.bfloat16, tag="mqs")
                        nc.scalar.activation(out=mqs[:], in_=mq[:], func=AF.Copy)
                        # pipelining: next sub's broadcast before this accum
                        if sub < 3:
                            mq_next = emit_bcast(cb, k, sub + 1)
                        elif k < KF - 1:
                            mq_next = emit_bcast(cb, k + 1, 0)
                        ht = htp.tile([128, 2048], DT.bfloat16, tag="ht")
                        nc.vector.tensor_tensor(
                            out=ht[:],
                            in0=mqs[:],
                            in1=gb[:, sub * 2048 : (sub + 1) * 2048],
                            op=ALU.mult)
                        # quad presum tree: 2048 -> 1024 -> 512
                        htv = ht[:]
                        s1 = vtp.tile([128, 1024], DT.bfloat16, tag="s1", name="s1")
                        nc.vector.tensor_tensor(
                            out=s1[:],
                            in0=bass.AP(htv.tensor, htv.offset, [htv.ap[0], [2, 1024]]),
                            in1=bass.AP(htv.tensor, htv.offset + 1, [htv.ap[0], [2, 1024]]),
                            op=ALU.add)
                        s1v = s1[:]
                        val = vtp.tile([128, 512], DT.bfloat16, tag="val", name="val")
                        nc.vector.tensor_tensor(
                            out=val[:],
                            in0=bass.AP(s1v.tensor, s1v.offset, [s1v.ap[0], [2, 512]]),
                            in1=bass.AP(s1v.tensor, s1v.offset + 1, [s1v.ap[0], [2, 512]]),
                            op=ALU.add)
                        nc.tensor.matmul(
                            out=po[:, sub * SUB : (sub + 1) * SUB],
                            lhsT=wdup[:, k * 128 : (k + 1) * 128],
                            rhs=val[:],
                            start=(k == 0), stop=(k == KF - 1),
                        )

                # bias add + stream->pixel unpermute, then store
                ot2 = otp.tile([128, MC], DT.float32, tag="ot")
                pov = po[:]
                nc.scalar.activation(
                    out=ot2[:],
                    in_=bass.AP(pov.tensor, pov.offset,
                                [pov.ap[0], [1, 16], [16, 128]]),
                    func=AF.Identity, bias=bdc_t[:, 0:1], scale=1.0)
                for h in range(2):
                    nc.sync.dma_start(
                        out=outv[:, h * NPIX + cb * MC : h * NPIX + (cb + 1) * MC],
                        in_=ot2[h * 64 : h * 64 + 64, :],
                    )


def _pack_params(w_om, b_om, w_dc, b_dc):
    # conv lhsT tiles [128, 9*54]: row h*64+c, col dd*54 + (role*18 + h*9 + k)
    lhs = np.zeros((128, KF * 54), np.float32)
    for dd in range(KF):
        dy, dx = dd // 3, dd % 3
        for h in range(2):
            for kk in range(KF):
                lhs[h * 64 : h * 64 + 64, dd * 54 + 0 * 18 + h * 9 + kk] = \
                    w_om[2 * kk, :, dy, dx]
                lhs[h * 64 : h * 64 + 64, dd * 54 + 1 * 18 + h * 9 + kk] = \
                    w_om[2 * kk + 1, :, dy, dx]
                lhs[h * 64 : h * 64 + 64, dd * 54 + 2 * 18 + h * 9 + kk] = \
                    w_om[18 + kk, :, dy, dx]

    # selector one-hots [72, 36*128]
    sel = np.zeros((MROW, KF * NCH * 128), np.float32)
    for cb in range(NCH):
        for k in range(KF):
            for p in range(128):
                h = p // 64
                sel[cb * 18 + h * 9 + k, (cb * KF + k) * 128 + p] = 1.0

    # wdup [128, 9*128] with the mask's 2.0 folded in
    wd = np.zeros((128, KF * 128), np.float32)
    for k in range(KF):
        kh, kw = k // 3, k % 3
        for h in range(2):
            wd[h * 64 : h * 64 + 64, k * 128 + h * 64 : k * 128 + h * 64 + 64] = \
                2.0 * w_dc[:, :, kh, kw].T

    # per-row constants [72, 4]: cst_y, cst_x, mask bias
    cstv = np.zeros((MROW, 4), np.float32)
    for p in range(MROW):
        cb, hk = divmod(p, 18)
        h, kk = divmod(hk, 9)
        kh, kw = kk // 3, kk % 3
        cstv[p, 0] = b_om[2 * kk] + cb * 16 + kh + 5
        cstv[p, 1] = b_om[2 * kk + 1] + kw + 1
        cstv[p, 2] = b_om[18 + kk]

    # iotas [72, 2*2048]: row (u//128) then col (u%128); exact in bf16
    u = np.arange(MC)
    iot = np.concatenate([u // 128, u % 128]).astype(np.float32)
    iot2 = np.broadcast_to(iot, (MROW, 2 * MC)).copy()

    bdc = np.zeros((128, 1), np.float32)
    bdc[:64, 0] = b_dc
    bdc[64:, 0] = b_dc

    return {
        "lhs_om": lhs.astype(BF16),
        "sel36": sel.astype(BF16),
        "wdup": wd.astype(BF16),
        "cst": cstv,
        "iot2": iot2.astype(BF16),
        "bdc_t": bdc,
    }


def _build_nc():
    _install_compat()
    nc = bass.Bass()
    ext = {}
    ext["x"] = nc.declare_dram_parameter("x", [C, H, W], DT.float32, isOutput=False)
    ext["lhs_om"] = nc.declare_dram_parameter("lhs_om", [128, KF * 54], DT.bfloat16, isOutput=False)
    ext["wdup"] = nc.declare_dram_parameter("wdup", [128, KF * 128], DT.bfloat16, isOutput=False)
    ext["sel36"] = nc.declare_dram_parameter("sel36", [MROW, KF * NCH * 128], DT.bfloat16, isOutput=False)
    ext["cst"] = nc.declare_dram_parameter("cst", [MROW, 4], DT.float32, isOutput=False)
    ext["iot2"] = nc.declare_dram_parameter("iot2", [MROW, 2 * MC], DT.bfloat16, isOutput=False)
    ext["bdc_t"] = nc.declare_dram_parameter("bdc_t", [128, 1], DT.float32, isOutput=False)
    ext["out"] = nc.declare_dram_parameter("out", [O, H, W], DT.float32, isOutput=True)
    with tile.TileContext(nc) as tc:
        _emit(nc, tc, ext)
    lower_extended_insts(nc)
    return nc


_NC_CACHE = None


def kernel(**inputs):
    global _NC_CACHE
    x = np.ascontiguousarray(inputs["x"], dtype=np.float32)
    w_om = np.ascontiguousarray(inputs["w_om"], dtype=np.float32)
    b_om = np.ascontiguousarray(inputs["b_om"], dtype=np.float32)
    w_dc = np.ascontiguousarray(inputs["w_dc"], dtype=np.float32)
    b_dc = np.ascontiguousarray(inputs["b_dc"], dtype=np.float32)

    if _NC_CACHE is None:
        _NC_CACHE = _build_nc()
    nc = _NC_CACHE

    packed = _pack_params(w_om, b_om, w_dc, b_dc)
    in_maps = [{"x": x[i], **packed} for i in range(NCORES)]
    res = run_bass_kernel_spmd(nc, in_maps, core_ids=list(range(NCORES)))
    return np.stack(
        [np.asarray(res.results[i]["out"]) for i in range(NCORES)]
    ).astype(np.float32)
